# revision 1
# baseline (speedup 1.0000x reference)
"""GraphSAGE mean-aggregation layer on 8 Trainium2 NeuronCores (raw Bass).

Math: out = D^{-1} A (x @ W + b)  ==  (D^{-1} A x) @ W + mask (outer) b
where A is the (row=dest, col=src) adjacency from edge_index, D = row degrees,
mask[d] = 1 if deg[d] > 0 else 0 (zero-degree rows are exactly 0 in the ref).

Strategy (one SPMD program on 8 cores, dest nodes sharded):
  - Host: sort edges by dest, bucket into 128-dest windows (wpc per core), pad
    each window to T tiles of 128 edges. Per-edge weight 1/deg[dest] is folded
    into the selection matrix so PSUM accumulation yields D^{-1}Ax directly.
  - Device, per window: one indirect-DMA gather of T*128 source rows (one row
    per partition per tile), then per 128-edge tile a DVE-built weighted
    one-hot S (S[e,j] = (dst_local[e]==j)*w[e]) and a PE matmul S^T @ G
    accumulating into PSUM [128 dests, 256]; transpose + W matmul + masked
    bias (K=1 outer product), DMA 128 output rows out.
  - Raw bass engine programs with explicit semaphores: this toolchain allows
    only ONE sync wait per instruction, so all waits are standalone wait_ge.
"""

import numpy as np

import concourse.bass as bass
import concourse.mybir as mybir
from concourse.bass_utils import run_bass_kernel_spmd

P = 128
F = 256

N_NODES = 100000
N_CORES = 8
NPC = N_NODES // N_CORES  # dest rows per core


def build_nc(n_nodes, npc, n_tiles, x_dtype=mybir.dt.float32, repeat=1):
    """One SPMD Bass program; n_tiles = edge tiles per 128-dest window."""
    wpc = (npc + P - 1) // P
    T = n_tiles
    f = F
    kf = f // P  # 2 feature chunks of 128
    NG = 2  # gather buffers
    dt_f32 = mybir.dt.float32

    nc = bass.Bass()

    x_h = nc.declare_dram_parameter("x", [n_nodes, f], x_dtype, isOutput=False)
    idx_h = nc.declare_dram_parameter("srcidx", [P, wpc * T], mybir.dt.int32, isOutput=False)
    dw_h = nc.declare_dram_parameter("dw", [P, wpc * 2 * T], dt_f32, isOutput=False)
    msk_h = nc.declare_dram_parameter("maskw", [wpc, P], dt_f32, isOutput=False)
    w_h = nc.declare_dram_parameter("Wm", [f, f], dt_f32, isOutput=False)
    b_h = nc.declare_dram_parameter("bv", [1, f], dt_f32, isOutput=False)
    out_h = nc.declare_dram_parameter("out", [npc, f], dt_f32, isOutput=True)

    NS = T + 12  # S-tile ring: one window + pipeline margin

    from contextlib import ExitStack

    ctx = ExitStack()
    with ctx:
        sb = lambda name, shape, dt: ctx.enter_context(nc.sbuf_tensor(name, shape, dt))
        ps = lambda name, shape: ctx.enter_context(nc.psum_tensor(name, shape, dt_f32))
        sem = lambda name: ctx.enter_context(nc.semaphore(name))

        iota_f = sb("iota_f", [P, P], dt_f32)
        ident = sb("ident", [P, P], dt_f32)
        w0 = sb("w0", [P, f], dt_f32)
        w1 = sb("w1", [P, f], dt_f32)
        b_sb = sb("b_sb", [1, f], dt_f32)
        idx_all = sb("idx_all", [P, wpc * T], mybir.dt.int32)
        dw_all = sb("dw_all", [P, wpc * 2 * T], dt_f32)
        msk_t = sb("msk_t", [1, 2 * P], dt_f32)
        g_buf = sb("g_buf", [P, NG * T * f], x_dtype)
        s_buf = sb("s_buf", [P, NS * P], x_dtype)
        agg_sb = sb("agg_sb", [P, 2 * f], dt_f32)
        tp_sb = sb("tp_sb", [P, kf * P], dt_f32)
        out_sb = sb("out_sb", [P, 2 * f], dt_f32)
        agg_ps = [ps("agg_ps0", [P, f]), ps("agg_ps1", [P, f])]
        tp_ps = [ps("tp_ps0", [P, P]), ps("tp_ps1", [P, P])]
        out_ps = [ps("out_ps0", [P, f]), ps("out_ps1", [P, f])]
        SEM_META = sem("sem_meta")
        SEM_CONST = sem("sem_const")
        SEM_G = sem("sem_g")
        SEM_S = sem("sem_s")
        SEM_MM = sem("sem_mm")
        SEM_CP = sem("sem_cp")
        SEM_TP = sem("sem_tp")
        SEM_TPC = sem("sem_tpc")
        SEM_FIN = sem("sem_fin")
        SEM_OUT = sem("sem_out")
        SEM_OD = sem("sem_od")
        SEM_MSK = sem("sem_msk")

        w_sb = [w0, w1]

        with nc.Block() as block:

            @block.sync
            def _(sync):
                # startup loads (HWDGE)
                sync.dma_start(w0[:, :], w_h[0:P, :]).then_inc(SEM_META, 16)
                sync.dma_start(w1[:, :], w_h[P : 2 * P, :]).then_inc(SEM_META, 16)
                sync.dma_start(b_sb[:, :], b_h[:, :]).then_inc(SEM_META, 16)
                sync.dma_start(idx_all[:, :], idx_h[:, :]).then_inc(SEM_META, 16)
                sync.dma_start(dw_all[:, :], dw_h[:, :]).then_inc(SEM_META, 16)
                # per-window mask loads + output stores
                for W in range(repeat * wpc):
                    w = W % wpc
                    rows = min(P, npc - w * P)
                    ob = (W % 2) * f
                    mb = (W % 2) * P
                    if W >= 2:
                        sync.wait_ge(SEM_FIN, W - 1)  # msk_t slot free
                    sync.dma_start(
                        msk_t[:1, mb : mb + P], msk_h[w : w + 1, :]
                    ).then_inc(SEM_MSK, 16)
                    sync.wait_ge(SEM_OUT, W + 1)
                    sync.dma_start(
                        out_h[w * P : w * P + rows, :], out_sb[:rows, ob : ob + f]
                    ).then_inc(SEM_OD, 16)

            @block.gpsimd
            def _(gpsimd):
                # constants
                gpsimd.iota(
                    iota_f[:, :],
                    pattern=[[1, P]],
                    base=0,
                    channel_multiplier=0,
                    allow_small_or_imprecise_dtypes=True,
                )
                gpsimd.memset(ident[:, :], 0.0)
                gpsimd.affine_select(
                    out=ident[:, :],
                    in_=ident[:, :],
                    compare_op=mybir.AluOpType.not_equal,
                    fill=1.0,
                    base=0,
                    pattern=[[-1, P]],
                    channel_multiplier=1,
                ).then_inc(SEM_CONST, 1)
                # gathers
                gpsimd.wait_ge(SEM_META, 80)
                for W in range(repeat * wpc):
                    w = W % wpc
                    gb = (W % NG) * T * f
                    if W >= NG:
                        # g buffer free once PE finished window W-NG's matmuls
                        gpsimd.wait_ge(SEM_MM, (W - NG + 1) * T)
                    for t in range(T):
                        # HW indirect DMA honors ONE offset per partition:
                        # one call per 128-edge tile.
                        gpsimd.indirect_dma_start(
                            out=g_buf[:, gb + t * f : gb + (t + 1) * f],
                            out_offset=None,
                            in_=x_h[:, :],
                            in_offset=bass.IndirectOffsetOnAxis(
                                ap=idx_all[:, w * T + t : w * T + t + 1], axis=0
                            ),
                        ).then_inc(SEM_G, 16)

            @block.vector
            def _(vector):
                vector.wait_ge(SEM_CONST, 1)
                vector.wait_ge(SEM_META, 80)
                for W in range(repeat * wpc):
                    w = W % wpc
                    # build S tiles for window w
                    for t in range(T):
                        i = W * T + t
                        sb = (i % NS) * P
                        if i >= NS:
                            vector.wait_ge(SEM_MM, i - NS + 1)
                        vector.tensor_scalar(
                            out=s_buf[:, sb : sb + P],
                            in0=iota_f[:, :],
                            scalar1=dw_all[:, w * 2 * T + t : w * 2 * T + t + 1],
                            scalar2=dw_all[:, w * 2 * T + T + t : w * 2 * T + T + t + 1],
                            op0=mybir.AluOpType.is_equal,
                            op1=mybir.AluOpType.mult,
                        ).then_inc(SEM_S, 1)
                    # copy window aggregate out of PSUM
                    ab = (W % 2) * f
                    vector.wait_ge(SEM_MM, (W + 1) * T)
                    vector.tensor_copy(
                        agg_sb[:, ab : ab + f], agg_ps[W % 2][:, :]
                    ).then_inc(SEM_CP, 1)
                    # copy transposes out of PSUM
                    for k in range(kf):
                        vector.wait_ge(SEM_TP, kf * W + k + 1)
                        vector.tensor_copy(
                            tp_sb[:, k * P : (k + 1) * P], tp_ps[k][:, :]
                        ).then_inc(SEM_TPC, 1)
                    # copy final output out of PSUM
                    ob = (W % 2) * f
                    if W >= 2:
                        vector.wait_ge(SEM_OD, (W - 1) * 16)
                    vector.wait_ge(SEM_FIN, W + 1)
                    vector.tensor_copy(
                        out_sb[:, ob : ob + f], out_ps[W % 2][:, :]
                    ).then_inc(SEM_OUT, 1)

            @block.tensor
            def _(tensor):
                tensor.wait_ge(SEM_META, 80)
                tensor.wait_ge(SEM_CONST, 1)
                for W in range(repeat * wpc):
                    w = W % wpc
                    ab = (W % 2) * f
                    gb = (W % NG) * T * f
                    if W >= 2:
                        tensor.wait_ge(SEM_CP, W - 1)  # agg bank free
                    tensor.wait_ge(SEM_S, (W + 1) * T)  # all S of window ready
                    for t in range(T):
                        i = W * T + t
                        sb = (i % NS) * P
                        tensor.wait_ge(SEM_G, 16 * (i + 1))  # tile t gathered
                        tensor.matmul(
                            agg_ps[W % 2][:, :],
                            s_buf[:, sb : sb + P],
                            g_buf[:, gb + t * f : gb + (t + 1) * f],
                            start=(t == 0),
                            stop=(t == T - 1),
                        ).then_inc(SEM_MM, 1)
                    tensor.wait_ge(SEM_CP, W + 1)  # agg_sb ready
                    for k in range(kf):
                        if W >= 1:
                            tensor.wait_ge(SEM_TPC, kf * (W - 1) + k + 1)  # tp bank free
                        tensor.transpose(
                            tp_ps[k][:, :],
                            agg_sb[:, ab + k * P : ab + (k + 1) * P],
                            ident[:, :],
                        ).then_inc(SEM_TP, 1)
                    ob = (W % 2) * f
                    if W >= 2:
                        tensor.wait_ge(SEM_OUT, W - 1)  # out_ps bank free
                    for k in range(kf):
                        tensor.wait_ge(SEM_TPC, kf * W + k + 1)  # tp_sb ready
                        tensor.matmul(
                            out_ps[W % 2][:, :],
                            tp_sb[:, k * P : (k + 1) * P],
                            w_sb[k][:, :],
                            start=(k == 0),
                            stop=False,
                        )
                    tensor.wait_ge(SEM_MSK, 16 * (W + 1))
                    tensor.matmul(
                        out_ps[W % 2][:, :],
                        msk_t[:1, (W % 2) * P : (W % 2) * P + P],
                        b_sb[:1, :],
                        start=False,
                        stop=True,
                    ).then_inc(SEM_FIN, 1)

    return nc


def prepare_inputs(x, edge_index, W, b, n_cores=N_CORES):
    """Host-side: sort/bucket edges by destination into per-core padded windows."""
    n = x.shape[0]
    npc = n // n_cores
    wpc = (npc + P - 1) // P

    row = np.asarray(edge_index[0], dtype=np.int64)  # dest
    col = np.asarray(edge_index[1], dtype=np.int64)  # src

    deg = np.bincount(row, minlength=n).astype(np.float32)
    invdeg = np.zeros(n, dtype=np.float32)
    nz = deg > 0
    invdeg[nz] = 1.0 / deg[nz]

    order = np.argsort(row, kind="stable")
    row_s = row[order]
    col_s = col[order]

    core_of = row_s // npc
    local = row_s - core_of * npc
    win = local // P
    dstl = local % P
    gwin = core_of * wpc + win
    n_gw = n_cores * wpc

    counts = np.bincount(gwin, minlength=n_gw)
    n_tiles = max(1, int(np.ceil(counts.max() / P)))
    T = n_tiles

    first = np.searchsorted(gwin, np.arange(n_gw))
    pos = np.arange(len(gwin)) - first[gwin]
    t_of = pos // P
    p_of = pos % P

    srcidx = np.zeros((n_cores, wpc, P, T), dtype=np.int32)
    dstloc = np.full((n_cores, wpc, P, 2 * T), -1.0, dtype=np.float32)

    srcidx[core_of, win, p_of, t_of] = col_s.astype(np.int32)
    dstloc[core_of, win, p_of, t_of] = dstl.astype(np.float32)
    dstloc[core_of, win, p_of, T + t_of] = invdeg[row_s]

    maskw = np.zeros((n_cores, wpc * P), dtype=np.float32)
    maskw[:, :npc] = nz.astype(np.float32).reshape(n_cores, npc)
    maskw = maskw.reshape(n_cores, wpc, P)

    x_c = np.ascontiguousarray(x, dtype=mybir.dt.np(mybir.dt.float32))
    per_core = []
    for c in range(n_cores):
        per_core.append(
            {
                "x": x_c,
                "srcidx": np.ascontiguousarray(
                    srcidx[c].transpose(1, 0, 2).reshape(P, wpc * T)
                ),
                "dw": np.ascontiguousarray(
                    dstloc[c].transpose(1, 0, 2).reshape(P, wpc * 2 * T)
                ),
                "maskw": maskw[c],
                "Wm": np.ascontiguousarray(W, dtype=np.float32),
                "bv": np.ascontiguousarray(b, dtype=np.float32).reshape(1, -1),
            }
        )
    return per_core, n_tiles


def run(x, edge_index, W, b, n_cores=N_CORES, trace=False):
    n, f = x.shape
    npc = n // n_cores
    in_maps, n_tiles = prepare_inputs(x, edge_index, W, b, n_cores)
    nc = build_nc(n, npc, n_tiles)
    res = run_bass_kernel_spmd(nc, in_maps, list(range(n_cores)), trace=trace)
    out = np.concatenate([res.results[c]["out"] for c in range(n_cores)], axis=0)
    return out, res


def kernel(x, edge_index, W, b):
    out, _ = run(np.asarray(x), np.asarray(edge_index), np.asarray(W), np.asarray(b))
    return out.astype(np.float32)



# revision 18
# speedup vs baseline: 1.1971x; 1.1971x over previous
"""GraphSAGE mean-aggregation layer on 8 Trainium2 NeuronCores (raw Bass).

Math: out = D^{-1} A (x @ W + b)  ==  (D^{-1} A x) @ W + mask (outer) b
where A is the (row=dest, col=src) adjacency from edge_index, D = row degrees,
mask[d] = 1 if deg[d] > 0 else 0 (zero-degree rows are exactly 0 in the ref).

Strategy (one SPMD program on 8 cores, dest nodes sharded):
  - Host: cast x to bf16, sort edges by dest, bucket into 128-dest windows,
    sub-bucket by source chunk (4 chunks of 25000 rows so local indices fit
    int16), pad each (window, chunk) segment to a multiple of 128 edges with
    the SAME padded size on every core (SPMD requires one instruction stream).
  - Device, per (window, chunk): ONE dma_gather ucode DMA fetches the whole
    segment (hundreds..thousands of rows) in a single Pool instruction —
    SWDGE fixed overhead (~1us) is paid per segment instead of per 128 rows
    as with indirect_dma_start, which was the old kernel's bottleneck.
  - Per 128-edge tile: DVE builds a weighted one-hot S (S[e,j] =
    (dst_local[e]==j) * (1/deg)) in bf16, PE accumulates S^T @ G into PSUM
    [128 dests, 256] in fp32. Epilogue per window: Act copies agg to SBUF
    (bf16), PE transposes + multiplies by W (bf16), adds mask (outer) b,
    Act copies the result out, HWDGE stores 128 output rows.
  - Raw bass engine programs with explicit semaphores: one sync wait per
    instruction, so all waits are standalone wait_ge.
"""

import numpy as np
import ml_dtypes

import concourse.bass as bass
import concourse.mybir as mybir
from concourse.bass_utils import run_bass_kernel_spmd

P = 128
F = 256

N_NODES = 100000
N_CORES = 8
NPC = N_NODES // N_CORES  # dest rows per core
WPC = (NPC + P - 1) // P  # 128-dest windows per core
NCHUNK = 4
CHUNK = 25000  # source rows per chunk (< 32768 so local idx fits int16)

BF16 = mybir.dt.bfloat16
F32 = mybir.dt.float32
I16 = mybir.dt.int16


def build_nc(seg_tiles, x_dtype=BF16):
    """One SPMD Bass program.

    seg_tiles: [WPC][NCHUNK] tiles per (window, chunk) segment (same on all
    cores; tiles*128 edges gathered per segment, 0 = segment skipped).
    """
    wpc = WPC
    f = F
    kf = f // P

    tiles_per_win = [sum(seg_tiles[w]) for w in range(wpc)]
    tmax = max(tiles_per_win)
    total_tiles = sum(tiles_per_win)
    total_cols = total_tiles * P // 16  # int16 idx columns
    # cumulative tiles before window w
    tcum = [0] * (wpc + 1)
    for w in range(wpc):
        tcum[w + 1] = tcum[w] + tiles_per_win[w]

    NW = 3  # g_buf ring depth in windows
    NS = 2 * tmax + 8  # s_buf ring depth in tiles
    MAXT = 8  # tiles per dma_gather (1024 idx; >1024 wedges the SWDGE ucode)

    # per window: list of sub-gathers (chunk, tile offset in window, ntiles);
    # idx columns follow the same order, contiguously
    win_gathers = []
    for w in range(wpc):
        gl = []
        toff = 0
        for k in range(NCHUNK):
            t = seg_tiles[w][k]
            for o in range(0, t, MAXT):
                gl.append((k, toff + o, min(MAXT, t - o)))
            toff += t
        win_gathers.append(gl)
    gcum = [0] * (wpc + 1)
    for w in range(wpc):
        gcum[w + 1] = gcum[w] + len(win_gathers[w])

    nc = bass.Bass()

    x_h = [
        nc.declare_dram_parameter(f"x{k}", [CHUNK, f], x_dtype, isOutput=False)
        for k in range(NCHUNK)
    ]
    idx_h = nc.declare_dram_parameter("srcidx", [P, total_cols], I16, isOutput=False)
    dw_h = nc.declare_dram_parameter("dw", [P, 2 * total_tiles], F32, isOutput=False)
    msk_h = nc.declare_dram_parameter("maskw", [1, wpc * P], BF16, isOutput=False)
    w_h = nc.declare_dram_parameter("Wm", [f, f], BF16, isOutput=False)
    b_h = nc.declare_dram_parameter("bv", [1, f], BF16, isOutput=False)
    iota_h = nc.declare_dram_parameter("iota", [P, P], BF16, isOutput=False)
    id_h = nc.declare_dram_parameter("ident", [P, P], BF16, isOutput=False)
    out_h = nc.declare_dram_parameter("out", [NPC, f], F32, isOutput=True)

    from contextlib import ExitStack

    ctx = ExitStack()
    with ctx:
        sb = lambda name, shape, dt: ctx.enter_context(nc.sbuf_tensor(name, shape, dt))
        ps = lambda name, shape, dt=F32: ctx.enter_context(nc.psum_tensor(name, shape, dt))
        sem = lambda name: ctx.enter_context(nc.semaphore(name))

        iota_f = sb("iota_f", [P, P], BF16)
        ident = sb("ident_sb", [P, P], BF16)
        w0 = sb("w0", [P, f], BF16)
        w1 = sb("w1", [P, f], BF16)
        b_sb = sb("b_sb", [1, f], BF16)
        msk_sb = sb("msk_sb", [1, wpc * P], BF16)
        idx_all = sb("idx_all", [P, total_cols], I16)
        dw_all = sb("dw_all", [P, 2 * total_tiles], F32)
        g_buf = sb("g_buf", [P, NW * tmax, f], x_dtype)
        s_buf = sb("s_buf", [P, NS, P], x_dtype)
        agg_sb = sb("agg_sb", [P, 2, f], BF16)
        tp_sb = sb("tp_sb", [P, 2, kf, P], BF16)
        out_sb = sb("out_sb", [P, 2, f], F32)
        agg_ps = [ps("agg_ps0", [P, f]), ps("agg_ps1", [P, f])]
        tp_ps = [ps("tp_ps0", [P, P], BF16), ps("tp_ps1", [P, P], BF16)]
        out_ps = [ps("out_ps0", [P, f]), ps("out_ps1", [P, f])]
        R = 8  # rotating gather-completion sems (SWDGE completions reorder)
        SEM_META = sem("sem_meta")
        SEM_G = [sem(f"sem_g{r}") for r in range(R)]
        SEM_S = sem("sem_s")
        SEM_MM = sem("sem_mm")
        SEM_CP = sem("sem_cp")  # agg PSUM->SBUF copies (1/window)
        SEM_TP = sem("sem_tp")  # transposes (kf/window)
        SEM_TPC = sem("sem_tpc")  # transpose copies (kf/window)
        SEM_FIN = sem("sem_fin")  # bias matmul done (1/window)
        SEM_OUT = sem("sem_out")  # out PSUM->SBUF copies (1/window)
        SEM_OD = [sem("sem_od0"), sem("sem_od1")]  # out DMA done (rotating)

        w_sb = [w0, w1]
        # startup loads: idx, dw, msk, Wx2, b, iota, ident = 8 DMAs
        N_LOADS = 8 * 16

        with nc.Block() as block:

            @block.sync
            def _(sync):
                sync.dma_start(idx_all[:, :], idx_h[:, :]).then_inc(SEM_META, 16)
                sync.dma_start(dw_all[:, :], dw_h[:, :]).then_inc(SEM_META, 16)
                sync.dma_start(msk_sb[:, :], msk_h[:, :]).then_inc(SEM_META, 16)
                sync.dma_start(w0[:, :], w_h[0:P, :]).then_inc(SEM_META, 16)
                sync.dma_start(w1[:, :], w_h[P : 2 * P, :]).then_inc(SEM_META, 16)
                sync.dma_start(b_sb[:, :], b_h[:, :]).then_inc(SEM_META, 16)
                sync.dma_start(iota_f[:, :], iota_h[:, :]).then_inc(SEM_META, 16)
                sync.dma_start(ident[:, :], id_h[:, :]).then_inc(SEM_META, 16)
                for w in range(wpc):
                    rows = min(P, NPC - w * P)
                    sync.wait_ge(SEM_OUT, w + 1)
                    sync.dma_start(
                        out_h[w * P : w * P + rows, :], out_sb[:rows, w % 2, :]
                    ).then_inc(SEM_OD[w % 2], 16)

            @block.gpsimd
            def _(gpsimd):
                gpsimd.wait_ge(SEM_META, N_LOADS)
                # one register per distinct gather size (to_reg per call
                # would exhaust the Pool register file)
                nidx_reg = {}
                for w in range(wpc):
                    for (_k, _o, nt) in win_gathers[w]:
                        if nt * P not in nidx_reg:
                            nidx_reg[nt * P] = gpsimd.to_reg(nt * P)
                col0 = 0
                j = 0  # gather ordinal
                for w in range(wpc):
                    first_of_window = True
                    for (k, toff, nt) in win_gathers[w]:
                        if first_of_window and w >= NW:
                            # g ring slot free once PE consumed window w-NW
                            gpsimd.wait_ge(SEM_MM, tcum[w - NW + 1])
                        first_of_window = False
                        if j >= R:
                            # sem slot reused: previous holder must be done
                            gpsimd.wait_ge(SEM_G[j % R], 16 * (j // R))
                        nidx = nt * P
                        cols = nidx // 16
                        base = (w % NW) * tmax + toff
                        gpsimd.dma_gather(
                            g_buf[:, base : base + nt, :],
                            x_h[k][:, :],
                            idx_all[:, col0 : col0 + cols],
                            nidx,
                            nidx_reg[nidx],
                            f,
                        ).then_inc(SEM_G[j % R], 16)
                        col0 += cols
                        j += 1

            @block.vector
            def _(vector):
                vector.wait_ge(SEM_META, N_LOADS)
                for w in range(wpc):
                    for t in range(tiles_per_win[w]):
                        i = tcum[w] + t
                        slot = i % NS
                        if i >= NS:
                            vector.wait_ge(SEM_MM, i - NS + 1)
                        vector.tensor_scalar(
                            out=s_buf[:, slot, :],
                            in0=iota_f[:, :],
                            scalar1=dw_all[:, 2 * i : 2 * i + 1],
                            scalar2=dw_all[:, 2 * i + 1 : 2 * i + 2],
                            op0=mybir.AluOpType.is_equal,
                            op1=mybir.AluOpType.mult,
                        ).then_inc(SEM_S, 1)

            @block.tensor
            def _(tensor):
                tensor.wait_ge(SEM_META, N_LOADS)
                # 3-stage software pipeline: agg matmuls of window w run
                # between the transposes of w-1 and the W-matmuls of w-2 so
                # PE never stalls on the Act-engine PSUM->SBUF copies.
                for step in range(wpc + 2):
                    w = step
                    if w < wpc:
                        T_w = tiles_per_win[w]
                        g_of_tile = []
                        g_ord = gcum[w]
                        for (_k, _o, nt) in win_gathers[w]:
                            g_ord += 1
                            g_of_tile += [g_ord] * nt
                        for t in range(T_w):
                            i = tcum[w] + t
                            slot = i % NS
                            if t == 0 and w >= 2:
                                tensor.wait_ge(SEM_CP, w - 1)  # agg bank free
                            if t == 0 or g_of_tile[t] != g_of_tile[t - 1]:
                                j = g_of_tile[t] - 1
                                tensor.wait_ge(SEM_G[j % R], 16 * (j // R + 1))
                            tensor.wait_ge(SEM_S, i + 1)
                            tensor.matmul(
                                agg_ps[w % 2][:, :],
                                s_buf[:, slot, :],
                                g_buf[:, (w % NW) * tmax + t, :],
                                start=(t == 0),
                                stop=(t == T_w - 1),
                            ).then_inc(SEM_MM, 1)
                    u = step - 2
                    if u >= 0:
                        # W matmuls + bias into out PSUM
                        for k in range(kf):
                            if k == 0 and u >= 2:
                                tensor.wait_ge(SEM_OUT, u - 1)  # out bank free
                            tensor.wait_ge(SEM_TPC, kf * u + k + 1)
                            tensor.matmul(
                                out_ps[u % 2][:, :],
                                tp_sb[:, u % 2, k, :],
                                w_sb[k][:, :],
                                start=(k == 0),
                                stop=False,
                            )
                        tensor.matmul(
                            out_ps[u % 2][:, :],
                            msk_sb[0:1, u * P : (u + 1) * P],
                            b_sb[0:1, :],
                            start=False,
                            stop=True,
                        ).then_inc(SEM_FIN, 1)
                    v = step - 1
                    if 0 <= v < wpc:
                        # transposes of agg (after Act copied agg to SBUF)
                        for k in range(kf):
                            tensor.wait_ge(SEM_CP, v + 1)
                            if v >= 1:
                                tensor.wait_ge(SEM_TPC, kf * (v - 1) + k + 1)
                            tensor.transpose(
                                tp_ps[k][:, :],
                                agg_sb[:, v % 2, k * P : (k + 1) * P],
                                ident[:, :],
                            ).then_inc(SEM_TP, 1)

            @block.scalar
            def _(scalar):
                scalar.wait_ge(SEM_META, N_LOADS)
                for w in range(wpc):
                    # agg PSUM -> SBUF (cast to bf16)
                    scalar.wait_ge(SEM_MM, tcum[w + 1])
                    if w >= 2:
                        scalar.wait_ge(SEM_TP, kf * (w - 1))  # agg_sb bank free
                    scalar.copy(agg_sb[:, w % 2, :], agg_ps[w % 2][:, :]).then_inc(
                        SEM_CP, 1
                    )
                    # transpose PSUM -> SBUF
                    for k in range(kf):
                        scalar.wait_ge(SEM_TP, kf * w + k + 1)
                        if w >= 2:
                            scalar.wait_ge(SEM_FIN, w - 1)  # tp_sb bank free
                        scalar.copy(tp_sb[:, w % 2, k, :], tp_ps[k][:, :]).then_inc(
                            SEM_TPC, 1
                        )
                    # final out PSUM -> SBUF (fp32)
                    scalar.wait_ge(SEM_FIN, w + 1)
                    if w >= 2:
                        # store w-2 (same sem slot) must be done before reuse
                        scalar.wait_ge(SEM_OD[w % 2], 16 * ((w - 2) // 2 + 1))
                    scalar.copy(out_sb[:, w % 2, :], out_ps[w % 2][:, :]).then_inc(
                        SEM_OUT, 1
                    )

    # extended-inst lowering (library loads + ISA bytes) — Bacc does this in
    # compile(); raw Bass must do it manually or dma_gather ships empty ISA.
    import bass_rust as _bass_rust
    from concourse.library_config import all_libraries, standard

    mask = {}
    for lib in all_libraries:
        for t in lib.instructions:
            mask[t] = mask.get(t, 0) | (1 << lib.index)
    _bass_rust.insert_library_loads(nc, mask, len(all_libraries), standard.index)
    mybir.codegen_inst_isa_subclasses(nc)
    return nc


def prepare_inputs(x, edge_index, W, b, n_cores=N_CORES):
    """Host-side: sort edges by dest, bucket per (core, window, src chunk)."""
    n = x.shape[0]
    npc = n // n_cores
    wpc = WPC

    row = np.asarray(edge_index[0], dtype=np.int64)  # dest
    col = np.asarray(edge_index[1], dtype=np.int64)  # src

    deg = np.bincount(row, minlength=n).astype(np.float32)
    invdeg = np.zeros(n, dtype=np.float32)
    nz = deg > 0
    invdeg[nz] = 1.0 / deg[nz]

    core_of = row // npc
    local = row - core_of * npc
    win = local // P
    dstl = (local % P).astype(np.float32)
    chunk = col // CHUNK
    src_local = (col - chunk * CHUNK).astype(np.int16)

    # segment key: (core, window, chunk); sort edges by it
    key = (core_of * wpc + win) * NCHUNK + chunk
    order = np.argsort(key, kind="stable")
    key_s = key[order]
    nseg = n_cores * wpc * NCHUNK
    cnt = np.bincount(key_s, minlength=nseg).reshape(n_cores, wpc, NCHUNK)

    # SPMD: same padded tile count on every core
    seg_tiles = np.ceil(cnt.max(axis=0) / P).astype(np.int64)  # [wpc, NCHUNK]
    # every window needs >= 1 tile so PSUM gets initialized
    empty_w = seg_tiles.sum(axis=1) == 0
    seg_tiles[empty_w, 0] = 1

    tiles_per_win = seg_tiles.sum(axis=1)  # [wpc]
    total_tiles = int(tiles_per_win.sum())
    total_slots = total_tiles * P
    total_cols = total_slots // 16

    # slot base per (window, chunk) in the per-core flat slot order
    seg_base = np.zeros((wpc, NCHUNK), dtype=np.int64)
    flat = seg_tiles.reshape(-1)
    seg_base.reshape(-1)[1:] = np.cumsum(flat)[:-1]
    seg_base *= 1  # in tiles
    seg_slot_base = seg_base * P

    # position of each edge within its (core, w, k) segment
    first = np.searchsorted(key_s, np.arange(nseg))
    pos = np.arange(len(key_s)) - first[key_s]

    c_s = key_s // (wpc * NCHUNK)
    wk_s = key_s % (wpc * NCHUNK)
    w_s = wk_s // NCHUNK
    k_s = wk_s % NCHUNK

    slot = seg_slot_base[w_s, k_s] + pos  # per-core flat slot

    srcidx = np.zeros((n_cores, 16, total_cols), dtype=np.int16)
    dw = np.zeros((n_cores, P, 2 * total_tiles), dtype=np.float32)

    src_s = src_local[order]
    dst_s = dstl[order]
    wgt_s = invdeg[row[order]]

    gtile = slot // P  # global tile index within core
    p128 = slot % P
    srcidx[c_s, (slot % P) % 16, (slot // P) * 8 + (slot % P) // 16] = src_s
    dw[c_s, p128, 2 * gtile] = dst_s
    dw[c_s, p128, 2 * gtile + 1] = wgt_s

    # idx slot i of a segment -> partition i%16, column (segment col base +
    # i//16). Column base of tile t = t*8. (Verified: within segment,
    # col = slot//16 relative to segment start; absolute = base_slots//16 +
    # pos//16. Since base_slots is a multiple of 128, (base+pos)//16 ==
    # base//16 + pos//16 only if pos%16 aligns -- base multiple of 16: yes.)

    maskw = np.zeros((n_cores, 1, wpc * P), dtype=ml_dtypes.bfloat16)
    nzr = nz.astype(np.float32).reshape(n_cores, npc)
    maskw[:, 0, :npc] = nzr.astype(ml_dtypes.bfloat16)

    x_bf = np.asarray(x, dtype=np.float32).astype(ml_dtypes.bfloat16)
    x_chunks = [
        np.ascontiguousarray(x_bf[k * CHUNK : (k + 1) * CHUNK]) for k in range(NCHUNK)
    ]

    iota = np.broadcast_to(
        np.arange(P, dtype=np.float32), (P, P)
    ).astype(ml_dtypes.bfloat16)
    ident_m = np.eye(P, dtype=np.float32).astype(ml_dtypes.bfloat16)
    W_bf = np.ascontiguousarray(np.asarray(W, dtype=np.float32)).astype(
        ml_dtypes.bfloat16
    )
    b_bf = (
        np.ascontiguousarray(np.asarray(b, dtype=np.float32))
        .reshape(1, -1)
        .astype(ml_dtypes.bfloat16)
    )

    per_core = []
    for c in range(n_cores):
        idx_full = np.zeros((P, total_cols), dtype=np.int16)
        for g in range(8):
            idx_full[16 * g : 16 * (g + 1), :] = srcidx[c]
        m = {
            "srcidx": idx_full,
            "dw": np.ascontiguousarray(dw[c]),
            "maskw": maskw[c],
            "Wm": W_bf,
            "bv": b_bf,
            "iota": np.ascontiguousarray(iota),
            "ident": ident_m,
        }
        for k in range(NCHUNK):
            m[f"x{k}"] = x_chunks[k]
        per_core.append(m)
    return per_core, seg_tiles.tolist()


def run(x, edge_index, W, b, n_cores=N_CORES, trace=False):
    in_maps, seg_tiles = prepare_inputs(x, edge_index, W, b, n_cores)
    nc = build_nc(seg_tiles)
    res = run_bass_kernel_spmd(nc, in_maps, list(range(n_cores)), trace=trace)
    out = np.concatenate([res.results[c]["out"] for c in range(n_cores)], axis=0)
    return out, res


def kernel(x, edge_index, W, b):
    out, _ = run(np.asarray(x), np.asarray(edge_index), np.asarray(W), np.asarray(b))
    return out.astype(np.float32)
